# revision 1
# baseline (speedup 1.0000x reference)
"""Ernie4 GQA attention layer as a Bass/Tile kernel for 8 TRN2 NeuronCores.

Sharding: core c = 4*b + g handles batch b (of 2) and head-group g (of 4).
Each group owns 8 query heads + 1 kv head (GQA 32q/4kv, head_dim 128) and the
matching column slice of w_qkv / row slice of w_o. The o_proj partial sums are
reduced on the host (all-reduce equivalent).

Per-core pipeline:
  phase 1 (token-major): qkv_tok = X^T.T @ W^T with the full 20MiB W slice
           cached in SBUF as the moving operand (N=512 f32r full rate) and
           X^T streamed exactly once as small stationary tiles.
  phase 2: per-head PE-transpose to feature-major, RoPE (swap-matmul + DVE),
           causal attention with transposed scores (scores^T[k, q]), exp on
           ACT, row-sums via a ones-matmul broadcast, PV accumulation in PSUM.
  phase 3: out_partial = ctx^T.T @ WoT (token-major psum, streamed WoT,
           tiled output layout un-tiled on host).

Feature order in wqkvt / qkv_tok scratch columns: [k, v, q0..q7].
"""
import sys

sys.path.insert(0, "/opt/trn_rl_repo")

import numpy as np

HIDDEN = 4096
N_Q_HEADS = 32
N_KV_HEADS = 4
HEAD_DIM = 128
ROPE_THETA = 500000.0
Q_SIZE = N_Q_HEADS * HEAD_DIM  # 4096
KV_SIZE = N_KV_HEADS * HEAD_DIM  # 512
B = 2
S = 2048
N_CORES = 8
N_GROUPS = 4
HEADS_PER_GROUP = N_Q_HEADS // N_GROUPS  # 8
GROUP_Q = HEADS_PER_GROUP * HEAD_DIM  # 1024
QKV_G = GROUP_Q + 2 * HEAD_DIM  # 1280 columns of qkv per group
SCALE = HEAD_DIM ** -0.5
NK = HIDDEN // 128  # 32 contraction k-tiles
NMT = QKV_G // 128  # 10 qkv feature tiles
NKT = S // 128  # 16 token/key tiles per sequence
NQB = S // 512  # 4 q-blocks
NHB = HIDDEN // 512  # 8 output-hidden blocks

_COMPILED = None
LAST_EXEC_NS = None


def _build(phases=(1, 2, 3)):
    import concourse.mybir as mybir
    import concourse.tile as tile
    from concourse import bacc

    F32 = mybir.dt.float32
    F32R = mybir.dt.float32r

    nc = bacc.Bacc("TRN2", target_bir_lowering=False, debug=False, num_devices=N_CORES)

    # xt tiled: [tt, 128, NK, 128]; xt[tt, h, ko, t] = X[tt*128+t, ko*128+h]
    # (hidden within k-tile on partitions, token within tile on free)
    xt = nc.dram_tensor("xt", [NKT, 128, NK, 128], F32R, kind="ExternalInput").ap()
    # wqkvt: [128, NK, 1280]; wqkvt[p, ko, f] = W^T[ko*128+p, f] (p = hidden)
    wqkvt = nc.dram_tensor("wqkvt", [128, NK, QKV_G], F32R, kind="ExternalInput").ap()
    wot = nc.dram_tensor("wot", [NHB, 128, HEADS_PER_GROUP, 512], F32R, kind="ExternalInput").ap()
    cos_t = nc.dram_tensor("cos_t", [HEAD_DIM, S], F32, kind="ExternalInput").ap()
    sin_t = nc.dram_tensor("sin_t", [HEAD_DIM, S], F32, kind="ExternalInput").ap()
    swp = nc.dram_tensor("swp", [128, 128], F32R, kind="ExternalInput").ap()
    ones = nc.dram_tensor("ones", [128, 128], F32R, kind="ExternalInput").ap()
    ident = nc.dram_tensor("ident", [128, 128], F32R, kind="ExternalInput").ap()
    maskt = nc.dram_tensor("maskt", [4, 128, 512], F32, kind="ExternalInput").ap()
    out_part = nc.dram_tensor(
        "out_part", [NKT, NHB, 128, 512], F32, kind="ExternalOutput"
    ).ap()

    with tile.TileContext(nc) as tc:
        with (
            tc.tile_pool(name="dram", bufs=1, space="DRAM") as dram,
        ):
            # token-major qkv scratch [tok, feat]; feature-major ctx scratch
            qkv_tok = dram.tile([S, QKV_G], F32R)
            ctx_scr = dram.tile([GROUP_Q, S], F32R)

            # ------- phase 1: qkv_tok[t, f] = sum_h X^T[h, t] W^T[h, f] -------
            with (
                tc.tile_pool(name="p1w", bufs=1) as p1w,
                tc.tile_pool(name="p1x", bufs=2) as p1x,
                tc.tile_pool(name="p1s", bufs=4) as p1s,
                tc.tile_pool(name="p1ps", bufs=5, space="PSUM") as p1ps,
            ):
                if 1 in phases:
                    w_all = p1w.tile([128, NK, QKV_G], F32R, tag="wall")
                    # chunked load so the first matmuls start early
                    for m in range(NMT):
                        nc.sync.dma_start(
                            w_all[:, :, m * 128:(m + 1) * 128],
                            wqkvt[:, :, m * 128:(m + 1) * 128],
                        )
                for tt in range(NKT if 1 in phases else 0):
                    x_tile = p1x.tile([128, NK, 128], F32R, tag="xtile")
                    nc.sync.dma_start(x_tile, xt[tt])
                    # fb blocks over features: [0:512), [512:1024), [1024:1280)
                    for fb in range(3):
                        f0 = fb * 512
                        fw = 512 if fb < 2 else 256
                        ps = p1ps.tile([128, 512], F32, tag="p1psum")
                        for k in range(NK):
                            nc.tensor.matmul(
                                ps[:, :fw],
                                x_tile[:, k, :],
                                w_all[:, k, f0:f0 + fw],
                                start=(k == 0),
                                stop=(k == NK - 1),
                            )
                        stage = p1s.tile([128, 512], F32R, tag="p1stage")
                        nc.any.tensor_copy(stage[:, :fw], ps[:, :fw])
                        nc.sync.dma_start(
                            qkv_tok[tt * 128:(tt + 1) * 128, f0:f0 + fw],
                            stage[:, :fw],
                        )

            # ------- phase 2: transpose to feature-major + RoPE + attention ---
            with (
                tc.tile_pool(name="p2c", bufs=1) as p2c,
                tc.tile_pool(name="kv", bufs=1) as kvpool,
                tc.tile_pool(name="qload", bufs=2) as qload,
                tc.tile_pool(name="qfeat", bufs=2) as qfeat,
                tc.tile_pool(name="rq", bufs=2) as rqpool,
                tc.tile_pool(name="ropet", bufs=4) as ropet,
                tc.tile_pool(name="pt", bufs=6) as ptpool,
                tc.tile_pool(name="ptm", bufs=4) as ptmpool,
                tc.tile_pool(name="rcp", bufs=3) as rcppool,
                tc.tile_pool(name="cstage", bufs=3) as cstage,
                tc.tile_pool(name="p2sc", bufs=5, space="PSUM") as p2sc,
                tc.tile_pool(name="p2r", bufs=1, space="PSUM") as p2r,
                tc.tile_pool(name="p2ctx", bufs=2, space="PSUM") as p2ctx,
            ):
                swp_sb = p2c.tile([128, 128], F32R)
                nc.sync.dma_start(swp_sb, swp)
                ones_sb = p2c.tile([128, 128], F32R)
                nc.sync.dma_start(ones_sb, ones)
                id_sb = p2c.tile([128, 128], F32R)
                nc.sync.dma_start(id_sb, ident)
                cos_sb = p2c.tile([128, S], F32)
                nc.sync.dma_start(cos_sb, cos_t)
                sin_sb = p2c.tile([128, S], F32)
                nc.sync.dma_start(sin_sb, sin_t)
                mask_sb = p2c.tile([128, 4, 512], F32)
                nc.sync.dma_start(mask_sb, maskt.rearrange("m p q -> p m q"))

                def load_tok(dst, col0):
                    # dst [128, NKT, 128] <- qkv_tok[:, col0:col0+128]
                    nc.sync.dma_start(
                        dst,
                        qkv_tok[:, col0:col0 + 128].rearrange(
                            "(tt p) d -> p tt d", p=128
                        ),
                    )

                def transpose_feat(dst, src_tok):
                    # dst [128, S] feature-major <- src_tok [128, NKT, 128]
                    for tt in range(NKT):
                        tps = p2sc.tile([128, 128], F32R, tag="scps")
                        nc.tensor.transpose(tps, src_tok[:, tt, :], id_sb)
                        nc.any.tensor_copy(dst[:, tt * 128:(tt + 1) * 128], tps)

                def rope(dst, src_tile):
                    # dst[f32r 128, S] = src*cos + (swap@src)*sin_signed
                    for c in range(S // 512):
                        cs = slice(c * 512, (c + 1) * 512)
                        sw_ps = p2sc.tile([128, 512], F32, tag="scps")
                        nc.tensor.matmul(
                            sw_ps, swp_sb, src_tile[:, cs], start=True, stop=True
                        )
                        t_sin = ropet.tile([128, 512], F32, tag="tsin")
                        nc.vector.tensor_mul(t_sin, sw_ps, sin_sb[:, cs])
                        t_cos = ropet.tile([128, 512], F32, tag="tcos")
                        nc.vector.tensor_mul(
                            t_cos, src_tile[:, cs].bitcast(F32), cos_sb[:, cs]
                        )
                        nc.vector.tensor_add(dst[:, cs], t_cos, t_sin)

                if 2 in phases:
                    k_tok = kvpool.tile([128, NKT, 128], F32R, tag="ktok")
                    load_tok(k_tok, 0)
                    k_feat = kvpool.tile([128, S], F32R, tag="kfeat")
                    transpose_feat(k_feat, k_tok)
                    rk = kvpool.tile([128, S], F32R, tag="rk")
                    rope(rk, k_feat)

                    vtok = kvpool.tile([128, NKT, 128], F32R, tag="vtok")
                    load_tok(vtok, 128)

                for h in range(HEADS_PER_GROUP if 2 in phases else 0):
                    q_tok = qload.tile([128, NKT, 128], F32R, tag="qtok")
                    load_tok(q_tok, 256 + h * 128)
                    qh = qfeat.tile([128, S], F32R, tag="qh")
                    transpose_feat(qh, q_tok)
                    rq = rqpool.tile([128, S], F32R, tag="rq")
                    rope(rq, qh)

                    for j in range(NQB):
                        qs = slice(j * 512, (j + 1) * 512)
                        nkt_j = 4 * (j + 1)  # causal: k-tiles 0..4j+3
                        ctx_ps = p2ctx.tile([128, 512], F32, tag="ctxps")
                        r_ps = p2r.tile([128, 512], F32, tag="rps")
                        for kt in range(nkt_j):
                            sc_ps = p2sc.tile([128, 512], F32, tag="scps")
                            nc.tensor.matmul(
                                sc_ps,
                                rk[:, kt * 128:(kt + 1) * 128],
                                rq[:, qs],
                                start=True,
                                stop=True,
                            )
                            pt = ptpool.tile([128, 512], F32R, tag="pt")
                            nc.scalar.activation(
                                pt, sc_ps,
                                mybir.ActivationFunctionType.Exp,
                                scale=SCALE,
                            )
                            if kt >= 4 * j:  # diagonal tile: causal mask
                                di = kt - 4 * j
                                ptm = ptmpool.tile([128, 512], F32R, tag="ptm")
                                nc.vector.tensor_mul(
                                    ptm, pt.bitcast(F32), mask_sb[:, di, :]
                                )
                                pt_use = ptm
                            else:
                                pt_use = pt
                            nc.tensor.matmul(
                                r_ps, ones_sb, pt_use,
                                start=(kt == 0), stop=(kt == nkt_j - 1),
                            )
                            nc.tensor.matmul(
                                ctx_ps, vtok[:, kt, :], pt_use,
                                start=(kt == 0), stop=(kt == nkt_j - 1),
                            )
                        rcp = rcppool.tile([128, 512], F32, tag="rcp")
                        nc.vector.reciprocal(rcp, r_ps)
                        cst = cstage.tile([128, 512], F32R, tag="cst")
                        nc.vector.tensor_mul(cst, ctx_ps, rcp)
                        nc.sync.dma_start(
                            ctx_scr[h * 128:(h + 1) * 128, qs], cst
                        )

            # ------- phase 3: out = ctx^T.T @ WoT -----------------------------
            with (
                tc.tile_pool(name="p3ctx", bufs=1) as p3ctx,
                tc.tile_pool(name="p3w", bufs=3) as p3w,
                tc.tile_pool(name="p3s", bufs=4) as p3s,
                tc.tile_pool(name="p3ps", bufs=4, space="PSUM") as p3ps,
            ):
                ctx_sb = p3ctx.tile([128, HEADS_PER_GROUP, S], F32R)
                for hh in range(HEADS_PER_GROUP):
                    nc.sync.dma_start(
                        ctx_sb[:, hh, :], ctx_scr[hh * 128:(hh + 1) * 128, :]
                    )
                for hb in range(NHB if 3 in phases else 0):
                    wo_blk = p3w.tile([128, HEADS_PER_GROUP, 512], F32R, tag="woblk")
                    nc.sync.dma_start(wo_blk, wot[hb])
                    for tt in range(NKT):
                        ps = p3ps.tile([128, 512], F32, tag="p3psum")
                        for hk in range(HEADS_PER_GROUP):
                            nc.tensor.matmul(
                                ps,
                                ctx_sb[:, hk, tt * 128:(tt + 1) * 128],
                                wo_blk[:, hk, :],
                                start=(hk == 0),
                                stop=(hk == HEADS_PER_GROUP - 1),
                            )
                        stage = p3s.tile([128, 512], F32, tag="p3stage")
                        nc.any.tensor_copy(stage, ps)
                        nc.sync.dma_start(out_part[tt, hb], stage)

    nc.compile()
    return nc


def _host_inputs(positions, hidden_states, w_qkv, w_o):
    """Shard + lay out inputs for the 8 cores (c = 4*b + g)."""
    positions = np.asarray(positions)
    hidden_states = np.asarray(hidden_states, dtype=np.float32)
    w_qkv = np.asarray(w_qkv, dtype=np.float32)
    w_o = np.asarray(w_o, dtype=np.float32)

    inv_freq = 1.0 / (ROPE_THETA ** (np.arange(0, HEAD_DIM, 2, dtype=np.float64) / HEAD_DIM))
    ang = positions.astype(np.float64)[None, :] * inv_freq[:, None]  # [half, S]
    cos_t = np.empty((HEAD_DIM, S), dtype=np.float32)
    sin_t = np.empty((HEAD_DIM, S), dtype=np.float32)
    c = np.cos(ang).astype(np.float32)
    s = np.sin(ang).astype(np.float32)
    cos_t[0::2] = c
    cos_t[1::2] = c
    sin_t[0::2] = -s
    sin_t[1::2] = s

    swp = np.zeros((128, 128), dtype=np.float32)
    idx = np.arange(0, 128, 2)
    swp[idx, idx + 1] = 1.0
    swp[idx + 1, idx] = 1.0
    ones = np.ones((128, 128), dtype=np.float32)
    ident = np.eye(128, dtype=np.float32)

    q_loc = np.arange(512)[None, :]
    k_loc = np.arange(128)[:, None]
    maskt = np.stack(
        [(q_loc - k_loc - 128 * di >= 0).astype(np.float32) for di in range(4)]
    )  # [4, 128, 512]

    # xt tiled: [NKT, 128, NK, 128]; xt_t[tt, hh, ko, t] = X[tt*128+t, ko*128+hh]
    # (hidden on partitions: lhsT tiles for the token-major qkv matmul)
    xts = []
    for b in range(B):
        xt_t = np.ascontiguousarray(
            hidden_states[b].reshape(NKT, 128, NK, 128).transpose(0, 3, 2, 1)
        )
        xts.append(xt_t)

    in_maps = []
    for c_id in range(N_CORES):
        b, g = divmod(c_id, N_GROUPS)
        cols = np.concatenate([
            np.arange(Q_SIZE + g * HEAD_DIM, Q_SIZE + (g + 1) * HEAD_DIM),  # k
            np.arange(Q_SIZE + KV_SIZE + g * HEAD_DIM, Q_SIZE + KV_SIZE + (g + 1) * HEAD_DIM),  # v
            np.arange(g * GROUP_Q, (g + 1) * GROUP_Q),  # q0..q7
        ])
        # wqkvt: [128, NK, 1280]; [p, ko, f] = w_qkv[cols[f], ko*128+p]
        wq = w_qkv[cols, :]  # [1280, 4096]
        wqkvt_t = np.ascontiguousarray(
            wq.T.reshape(NK, 128, QKV_G).transpose(1, 0, 2)
        )
        wot_full = w_o[:, g * GROUP_Q:(g + 1) * GROUP_Q].T  # [1024, 4096]
        wot_t = np.ascontiguousarray(
            wot_full.reshape(HEADS_PER_GROUP, 128, NHB, 512).transpose(2, 1, 0, 3)
        )
        in_maps.append({
            "xt": xts[b],
            "wqkvt": wqkvt_t,
            "wot": wot_t,
            "cos_t": cos_t,
            "sin_t": sin_t,
            "swp": swp,
            "ones": ones,
            "ident": ident,
            "maskt": maskt,
        })
    return in_maps


def kernel(positions, hidden_states, w_qkv, w_o):
    global _COMPILED, LAST_EXEC_NS
    from concourse import bass_utils

    if _COMPILED is None:
        _COMPILED = _build()
    nc = _COMPILED

    in_maps = _host_inputs(positions, hidden_states, w_qkv, w_o)
    res = bass_utils.run_bass_kernel_spmd(
        nc, in_maps, core_ids=list(range(N_CORES))
    )
    LAST_EXEC_NS = res.exec_time_ns

    out = np.zeros((B, S, HIDDEN), dtype=np.float32)
    for c_id in range(N_CORES):
        b = c_id // N_GROUPS
        part = res.results[c_id]["out_part"]  # [NKT, NHB, 128, 512]
        out[b] += part.transpose(0, 2, 1, 3).reshape(S, HIDDEN)
    return out



# revision 8
# speedup vs baseline: 1.2638x; 1.2638x over previous
"""Ernie4 GQA attention layer as a Bass/Tile kernel for 8 TRN2 NeuronCores (v3).

Sharding: core c = 4*b + g handles batch b (of 2) and head-group g (of 4).
Each group owns 8 query heads + 1 kv head (GQA 32q/4kv, head_dim 128) and the
matching column slice of w_qkv / row slice of w_o. The o_proj partial sums are
reduced on the host (all-reduce equivalent).

v3 design: everything SBUF-resident in bf16 (PE rate equals f32r at 1
cycle/row but SBUF/DMA halve), no DRAM scratch round-trips.
  p1: qkv = X^T.T @ W^T token-major, 3 feature blocks (kv 256 | q 512 | q 512)
      per token tile. X rows loaded as contiguous 8KB lines; W chunk DMAs
      interleaved with the first X tiles so the PE starts after ~4us.
      Per-head PE transposes interleaved into the p1 loop.
  p2: RoPE via PSUM accumulation: q' = swp@(q*sin') + I@(q*cos) — two cheap
      bf16 DVE muls + two PE matmuls + one ACT copy per 512-chunk (no serial
      3-op DVE chain). RoPE of head h+1 is interleaved into attention(j=0,h).
  attention j-outer/h-inner, transposed scores; exp on ACT -> pt bf16;
      softmax denominators accumulated in TWO independent accumulators
      (Pool-chain + DVE-chain) finished by two PSUM-accumulating
      ones-matmuls; causal diagonal tiles narrowed to valid columns.
  p3: o_proj matmuls for q-block j-1 interleaved into attention of q-block j
      (PE executes in program order: they fill ACT/Pool latency bubbles),
      ctx in SBUF bf16, w_o resident bf16.
"""
import sys

sys.path.insert(0, "/opt/trn_rl_repo")

import numpy as np

HIDDEN = 4096
N_Q_HEADS = 32
N_KV_HEADS = 4
HEAD_DIM = 128
ROPE_THETA = 500000.0
Q_SIZE = N_Q_HEADS * HEAD_DIM  # 4096
KV_SIZE = N_KV_HEADS * HEAD_DIM  # 512
B = 2
S = 2048
N_CORES = 8
N_GROUPS = 4
HEADS_PER_GROUP = N_Q_HEADS // N_GROUPS  # 8
GROUP_Q = HEADS_PER_GROUP * HEAD_DIM  # 1024
QKV_G = GROUP_Q + 2 * HEAD_DIM  # 1280 columns of qkv per group
SCALE = HEAD_DIM ** -0.5
NK = HIDDEN // 128  # 32 contraction k-tiles
NKT = S // 128  # 16 token/key tiles per sequence
NQB = S // 512  # 4 q-blocks
NHB = HIDDEN // 512  # 8 output-hidden blocks

_COMPILED = None
LAST_EXEC_NS = None


def _build(phases=(1, 2, 3)):
    import concourse.mybir as mybir
    import concourse.tile as tile
    from concourse import bacc

    F32 = mybir.dt.float32
    F32R = mybir.dt.float32r
    BF16 = mybir.dt.bfloat16

    nc = bacc.Bacc("TRN2", target_bir_lowering=False, debug=False, num_devices=N_CORES)

    # xt: [tt, 128, NK*128]; xt[tt, h, ko*128+t] = X[tt*128+t, ko*128+h]
    # (contiguous 8KB per partition row -> single-descriptor DMA lines)
    xt = nc.dram_tensor("xt", [NKT, 128, NK * 128], BF16, kind="ExternalInput").ap()
    # wqkvt: [128, NK, 1280]; wqkvt[p, ko, f] = W^T[ko*128+p, f], f=[k,v,q0..q7]
    wqkvt = nc.dram_tensor("wqkvt", [128, NK, QKV_G], BF16, kind="ExternalInput").ap()
    # wot: [128, hk, 4096]; wot[p, hk, o] = w_o[o, g*1024 + hk*128 + p]
    wot = nc.dram_tensor("wot", [128, HEADS_PER_GROUP, HIDDEN], BF16,
                         kind="ExternalInput").ap()
    # rope tables (bf16): cos2[2i]=cos2[2i+1]=cos; sin2[2i]=+sin, sin2[2i+1]=-sin
    cos2 = nc.dram_tensor("cos2", [HEAD_DIM, S], BF16, kind="ExternalInput").ap()
    sin2 = nc.dram_tensor("sin2", [HEAD_DIM, S], BF16, kind="ExternalInput").ap()
    swp = nc.dram_tensor("swp", [128, 128], BF16, kind="ExternalInput").ap()
    ones = nc.dram_tensor("ones", [128, 128], F32R, kind="ExternalInput").ap()
    ident = nc.dram_tensor("ident", [128, 128], BF16, kind="ExternalInput").ap()
    # triangular edge mask: maskt[p, c] = 1 if c >= p else 0
    maskt = nc.dram_tensor("maskt", [128, 128], BF16, kind="ExternalInput").ap()
    out_part = nc.dram_tensor(
        "out_part", [NKT, NHB, 128, 512], F32, kind="ExternalOutput"
    ).ap()

    FB = ((0, 256), (256, 512), (768, 512))  # (col0, width): kv | q0..3 | q4..7

    with tile.TileContext(nc) as tc:
        with (
            tc.tile_pool(name="consts", bufs=1) as cpool,
            tc.tile_pool(name="kvsb", bufs=1) as kvpool,
            tc.tile_pool(name="rqsb", bufs=1) as rqpool,
        ):
            swp_sb = cpool.tile([128, 128], BF16)
            nc.sync.dma_start(swp_sb, swp)
            ones_sb = cpool.tile([128, 128], F32R)
            nc.sync.dma_start(ones_sb, ones)
            id_sb = cpool.tile([128, 128], BF16)
            nc.sync.dma_start(id_sb, ident)
            mask_sb = cpool.tile([128, 128], BF16)
            nc.sync.dma_start(mask_sb, maskt)
            cos_sb = cpool.tile([128, S], BF16)
            sin_sb = cpool.tile([128, S], BF16)

            # persistent SBUF state
            kv_sb = [kvpool.tile([128, 256], BF16, name=f"kv{i}") for i in range(NKT)]
            rq = [rqpool.tile([128, S], BF16, name=f"rq{i}") for i in range(HEADS_PER_GROUP)]
            rk = rqpool.tile([128, S], BF16)

            # ---------------- phase 1: qkv + interleaved transposes ---------
            with (
                tc.tile_pool(name="w", bufs=1) as wpool,
                tc.tile_pool(name="x", bufs=3) as xpool,
                tc.tile_pool(name="qs", bufs=3) as qspool,
                tc.tile_pool(name="p1ps", bufs=3, space="PSUM") as p1ps,
                tc.tile_pool(name="tp", bufs=4, space="PSUM") as tpps,
            ):
                w_tiles = {}
                x_tiles = {}

                def emit_x(tt):
                    x_tile = xpool.tile([128, NK * 128], BF16, tag="xt", name="xtl")
                    nc.sync.dma_start(x_tile, xt[tt])
                    x_tiles[tt] = x_tile

                if 1 in phases:
                    # first W chunk, then x0, then the rest interleaved so the
                    # serial DMA device serves the critical path first
                    for fb, (c0, fw) in enumerate(FB):
                        for kc in range(4):
                            wt = wpool.tile([128, 8, fw], BF16, name=f"w{fb}_{kc}")
                            w_tiles[(fb, kc)] = wt

                    def wdma(fb, kc):
                        c0, fw = FB[fb]
                        nc.sync.dma_start(
                            w_tiles[(fb, kc)],
                            wqkvt[:, kc * 8:(kc + 1) * 8, c0:c0 + fw],
                        )

                    wdma(0, 0)
                    emit_x(0)
                    wdma(0, 1), wdma(0, 2), wdma(0, 3)
                    emit_x(1)
                    for kc in range(4):
                        wdma(1, kc)
                    emit_x(2)
                    for kc in range(4):
                        wdma(2, kc)
                    # rope tables can trickle in behind the p1-critical DMAs
                    nc.sync.dma_start(cos_sb, cos2)
                    nc.sync.dma_start(sin_sb, sin2)

                def transpose_to(src):
                    tps = tpps.tile([128, 128], BF16, tag="tp", name="tps")
                    nc.tensor.transpose(tps, src, id_sb)
                    return tps

                pend_q = []  # deferred q transposes: (qs_tile, hh_base, tt)

                for tt in range(NKT if 1 in phases else 0):
                    if tt >= 3 and tt < NKT:
                        pass
                    for fb, (c0, fw) in enumerate(FB):
                        ps = p1ps.tile([128, 512], F32, tag="p1", name="p1t")
                        for k in range(NK):
                            nc.tensor.matmul(
                                ps[:, :fw],
                                x_tiles[tt][:, k * 128:(k + 1) * 128],
                                w_tiles[(fb, k // 8)][:, k % 8, :],
                                start=(k == 0),
                                stop=(k == NK - 1),
                            )
                        if fb == 0:
                            nc.vector.tensor_copy(kv_sb[tt], ps[:, :256])
                            if tt + 3 < NKT:
                                emit_x(tt + 3)
                        else:
                            qs = qspool.tile([128, 512], BF16, tag="qs", name="qst")
                            nc.vector.tensor_copy(qs, ps[:, :fw])
                            pend_q.append((qs, (fb - 1) * 4, tt))
                        if fb == 1:
                            # k transpose emitted while fb2 matmuls fill PE
                            tps = transpose_to(kv_sb[tt][:, 0:128])
                            nc.scalar.copy(rk[:, tt * 128:(tt + 1) * 128], tps)
                        # drain one pending q-transpose batch per block,
                        # lagging behind the DVE copies
                        if len(pend_q) > 1:
                            qs_t, hh0, qtt = pend_q.pop(0)
                            for hh in range(4):
                                h = hh0 + hh
                                tps = transpose_to(qs_t[:, hh * 128:(hh + 1) * 128])
                                nc.scalar.copy(
                                    rq[h][:, qtt * 128:(qtt + 1) * 128], tps
                                )
                for qs_t, hh0, qtt in pend_q:
                    for hh in range(4):
                        h = hh0 + hh
                        tps = transpose_to(qs_t[:, hh * 128:(hh + 1) * 128])
                        nc.scalar.copy(rq[h][:, qtt * 128:(qtt + 1) * 128], tps)

            # ---------------- phase 2/3: rope + attention + o_proj ----------
            with (
                tc.tile_pool(name="wo", bufs=1) as wopool,
                tc.tile_pool(name="ropet", bufs=4) as ropet,
                tc.tile_pool(name="pt", bufs=6) as ptpool,
                tc.tile_pool(name="pacc", bufs=4) as paccpool,
                tc.tile_pool(name="rcp", bufs=2) as rcppool,
                tc.tile_pool(name="ctxsb", bufs=2) as ctxsbpool,
                tc.tile_pool(name="ost", bufs=4) as ostpool,
                tc.tile_pool(name="scps", bufs=3, space="PSUM") as scps,
                tc.tile_pool(name="ctxps", bufs=2, space="PSUM") as ctxps,
                tc.tile_pool(name="opps", bufs=2, space="PSUM") as opps,
                tc.tile_pool(name="rps", bufs=1, space="PSUM") as rpsp,
            ):
                wo_sb = wopool.tile([128, HEADS_PER_GROUP, HIDDEN], BF16)
                if 3 in phases:
                    for hb in range(NHB):
                        nc.sync.dma_start(
                            wo_sb[:, :, hb * 512:(hb + 1) * 512],
                            wot[:, :, hb * 512:(hb + 1) * 512],
                        )

                def rope_inplace(t):
                    # t[:, cs] = swp@(t*sin') + I@(t*cos), psum-accumulated
                    for c in range(S // 512):
                        cs = slice(c * 512, (c + 1) * 512)
                        m_sin = ropet.tile([128, 512], BF16, tag="msin", name="msin")
                        nc.vector.tensor_mul(m_sin, t[:, cs], sin_sb[:, cs])
                        m_cos = ropet.tile([128, 512], BF16, tag="mcos", name="mcos")
                        nc.vector.tensor_mul(m_cos, t[:, cs], cos_sb[:, cs])
                        ps = scps.tile([128, 512], F32, tag="sc", name="ropeps")
                        nc.tensor.matmul(ps, swp_sb, m_sin, start=True, stop=False)
                        nc.tensor.matmul(ps, id_sb, m_cos, start=False, stop=True)
                        nc.scalar.copy(t[:, cs], ps)

                if 2 in phases:
                    rope_inplace(rk)
                    rope_inplace(rq[0])

                # o_proj work items for q-block j, emitted interleaved during
                # attention of q-block j+1 (fills PE while ACT/Pool run)
                class OpEmitter:
                    def __init__(self, j, ctx_sb_j):
                        self.items = [
                            (tl, hb, hk)
                            for tl in range(4)
                            for hb in range(NHB)
                            for hk in range(HEADS_PER_GROUP)
                        ] if (3 in phases) else []
                        self.j = j
                        self.ctx = ctx_sb_j
                        self.pos = 0
                        self.ps = None

                    def emit(self, n):
                        for _ in range(n):
                            if self.pos >= len(self.items):
                                return
                            tl, hb, hk = self.items[self.pos]
                            if hk == 0:
                                self.ps = opps.tile([128, 512], F32, tag="op", name="opps")
                            nc.tensor.matmul(
                                self.ps,
                                self.ctx[:, hk, tl * 128:(tl + 1) * 128],
                                wo_sb[:, hk, hb * 512:(hb + 1) * 512],
                                start=(hk == 0),
                                stop=(hk == HEADS_PER_GROUP - 1),
                            )
                            if hk == HEADS_PER_GROUP - 1:
                                ost = ostpool.tile([128, 512], F32, tag="ost", name="ost")
                                nc.vector.tensor_copy(ost, self.ps)
                                nc.sync.dma_start(
                                    out_part[self.j * 4 + tl, hb], ost
                                )
                            self.pos += 1

                    def flush(self):
                        self.emit(len(self.items) - self.pos)

                prev_op = None
                for j in range(NQB if 2 in phases else 0):
                    nkt_j = 4 * (j + 1)
                    # diagonal tiles first (descending width), then full tiles
                    kt_order = list(range(4 * j, 4 * j + 4)) + list(range(4 * j))
                    ctx_sb_j = ctxsbpool.tile(
                        [128, HEADS_PER_GROUP, 512], BF16, tag="ctx", name="ctxsb"
                    )
                    op_rate = -(-32 // nkt_j) if prev_op is not None else 0
                    for h in range(HEADS_PER_GROUP):
                        ctx_ps = ctxps.tile([128, 512], F32, tag="ctxp", name="ctxpt")
                        pacc_a = paccpool.tile([128, 512], F32R, tag="pacca", name="pacca")
                        pacc_b = None
                        b_init = False
                        flip = False

                        def sc_emit(i):
                            kt = kt_order[i]
                            di = kt - 4 * j
                            col0 = di * 128 if di >= 0 else 0
                            sc_ps = scps.tile([128, 512], F32, tag="sc", name="scpst")
                            nc.tensor.matmul(
                                sc_ps[:, col0:],
                                rk[:, kt * 128:(kt + 1) * 128],
                                rq[h][:, j * 512 + col0:(j + 1) * 512],
                                start=True,
                                stop=True,
                            )
                            return sc_ps, kt, col0

                        pend = [sc_emit(0)]
                        if nkt_j > 1:
                            pend.append(sc_emit(1))
                        for i in range(nkt_j):
                            sc_ps, kt, col0 = pend.pop(0)
                            di = kt - 4 * j
                            pt = ptpool.tile([128, 512], BF16, tag="pt", name="ptt")
                            nc.scalar.activation(
                                pt[:, col0:], sc_ps[:, col0:],
                                mybir.ActivationFunctionType.Exp,
                                scale=SCALE,
                            )
                            if di >= 0:  # diagonal: mask triangular edge
                                nc.vector.tensor_mul(
                                    pt[:, col0:col0 + 128],
                                    pt[:, col0:col0 + 128],
                                    mask_sb,
                                )
                            # dual-chain denominator accumulation
                            if i == 0:
                                nc.gpsimd.tensor_copy(pacc_a, pt)
                            elif col0 == 0 and not b_init:
                                pacc_b = paccpool.tile(
                                    [128, 512], F32R, tag="paccb", name="paccb"
                                )
                                nc.vector.tensor_copy(pacc_b, pt)
                                b_init = True
                            elif (not b_init) or flip:
                                nc.gpsimd.tensor_add(
                                    pacc_a[:, col0:],
                                    pacc_a[:, col0:].bitcast(F32),
                                    pt[:, col0:],
                                )
                                flip = False
                            else:
                                nc.vector.tensor_add(
                                    pacc_b[:, col0:],
                                    pacc_b[:, col0:].bitcast(F32),
                                    pt[:, col0:],
                                )
                                flip = True
                            nc.tensor.matmul(
                                ctx_ps[:, col0:],
                                kv_sb[kt][:, 128:256],
                                pt[:, col0:],
                                start=(i == 0),
                                stop=(i == nkt_j - 1),
                                skip_group_check=True,
                            )
                            if prev_op is not None:
                                prev_op.emit(op_rate)
                            if i + 2 < nkt_j:
                                pend.append(sc_emit(i + 2))
                        # softmax denominator: partition-sum of the chains
                        r_ps = rpsp.tile([128, 512], F32, tag="rps", name="rpst")
                        nc.tensor.matmul(
                            r_ps, ones_sb, pacc_a,
                            start=True, stop=not b_init,
                        )
                        if b_init:
                            nc.tensor.matmul(
                                r_ps, ones_sb, pacc_b,
                                start=False, stop=True,
                            )
                        rcp = rcppool.tile([128, 512], F32, tag="rcp", name="rcpt")
                        nc.vector.reciprocal(rcp, r_ps)
                        nc.vector.tensor_mul(ctx_sb_j[:, h, :], ctx_ps, rcp)
                        # interleave rope of the next head into attention(j=0)
                        if j == 0 and h + 1 < HEADS_PER_GROUP:
                            rope_inplace(rq[h + 1])
                    if prev_op is not None:
                        prev_op.flush()
                    prev_op = OpEmitter(j, ctx_sb_j)
                if prev_op is not None:
                    prev_op.flush()

    nc.compile()
    return nc


def _host_inputs(positions, hidden_states, w_qkv, w_o):
    """Shard + lay out inputs for the 8 cores (c = 4*b + g)."""
    import ml_dtypes

    bf16 = ml_dtypes.bfloat16
    positions = np.asarray(positions)
    hidden_states = np.asarray(hidden_states, dtype=np.float32)
    w_qkv = np.asarray(w_qkv, dtype=np.float32)
    w_o = np.asarray(w_o, dtype=np.float32)

    inv_freq = 1.0 / (ROPE_THETA ** (np.arange(0, HEAD_DIM, 2, dtype=np.float64) / HEAD_DIM))
    ang = positions.astype(np.float64)[None, :] * inv_freq[:, None]  # [half, S]
    c = np.cos(ang).astype(np.float32)
    s = np.sin(ang).astype(np.float32)
    cos2 = np.empty((HEAD_DIM, S), dtype=np.float32)
    sin2 = np.empty((HEAD_DIM, S), dtype=np.float32)
    cos2[0::2] = c
    cos2[1::2] = c
    sin2[0::2] = s
    sin2[1::2] = -s

    swp = np.zeros((128, 128), dtype=np.float32)
    idx = np.arange(0, 128, 2)
    swp[idx, idx + 1] = 1.0
    swp[idx + 1, idx] = 1.0
    ones = np.ones((128, 128), dtype=np.float32)
    ident = np.eye(128, dtype=np.float32)
    maskt = (np.arange(128)[None, :] >= np.arange(128)[:, None]).astype(np.float32)

    xts = []
    for b in range(B):
        xt_t = np.ascontiguousarray(
            hidden_states[b].reshape(NKT, 128, NK, 128).transpose(0, 3, 2, 1)
        ).reshape(NKT, 128, NK * 128).astype(bf16)
        xts.append(xt_t)

    in_maps = []
    for c_id in range(N_CORES):
        b, g = divmod(c_id, N_GROUPS)
        cols = np.concatenate([
            np.arange(Q_SIZE + g * HEAD_DIM, Q_SIZE + (g + 1) * HEAD_DIM),  # k
            np.arange(Q_SIZE + KV_SIZE + g * HEAD_DIM, Q_SIZE + KV_SIZE + (g + 1) * HEAD_DIM),  # v
            np.arange(g * GROUP_Q, (g + 1) * GROUP_Q),  # q0..q7
        ])
        wq = w_qkv[cols, :]  # [1280, 4096]
        wqkvt_t = np.ascontiguousarray(
            wq.T.reshape(NK, 128, QKV_G).transpose(1, 0, 2)
        ).astype(bf16)
        wot_full = w_o[:, g * GROUP_Q:(g + 1) * GROUP_Q].T  # [1024, 4096]
        wot_t = np.ascontiguousarray(
            wot_full.reshape(HEADS_PER_GROUP, 128, HIDDEN).transpose(1, 0, 2)
        ).astype(bf16)
        in_maps.append({
            "xt": xts[b],
            "wqkvt": wqkvt_t,
            "wot": wot_t,
            "cos2": cos2.astype(bf16),
            "sin2": sin2.astype(bf16),
            "swp": swp.astype(bf16),
            "ones": ones,
            "ident": ident.astype(bf16),
            "maskt": maskt.astype(bf16),
        })
    return in_maps


def kernel(positions, hidden_states, w_qkv, w_o):
    global _COMPILED, LAST_EXEC_NS
    from concourse import bass_utils

    if _COMPILED is None:
        _COMPILED = _build()
    nc = _COMPILED

    in_maps = _host_inputs(positions, hidden_states, w_qkv, w_o)
    res = bass_utils.run_bass_kernel_spmd(
        nc, in_maps, core_ids=list(range(N_CORES))
    )
    LAST_EXEC_NS = res.exec_time_ns

    out = np.zeros((B, S, HIDDEN), dtype=np.float32)
    for c_id in range(N_CORES):
        b = c_id // N_GROUPS
        part = res.results[c_id]["out_part"]  # [NKT, NHB, 128, 512]
        out[b] += part.transpose(0, 2, 1, 3).reshape(S, HIDDEN)
    return out


# revision 15
# speedup vs baseline: 1.3378x; 1.0585x over previous
"""Ernie4 GQA attention layer as a Bass/Tile kernel for 8 TRN2 NeuronCores (v3).

Sharding: core c = 4*b + g handles batch b (of 2) and head-group g (of 4).
Each group owns 8 query heads + 1 kv head (GQA 32q/4kv, head_dim 128) and the
matching column slice of w_qkv / row slice of w_o. The o_proj partial sums are
reduced on the host (all-reduce equivalent).

v3 design: everything SBUF-resident in bf16 (PE rate equals f32r at 1
cycle/row but SBUF/DMA halve), no DRAM scratch round-trips.
  p1: qkv = X^T.T @ W^T token-major, 3 feature blocks (kv 256 | q 512 | q 512)
      per token tile. X rows loaded as contiguous 8KB lines; W chunk DMAs
      interleaved with the first X tiles so the PE starts after ~4us.
      Per-head PE transposes interleaved into the p1 loop.
  p2: RoPE via PSUM accumulation: q' = swp@(q*sin') + I@(q*cos) — two cheap
      bf16 DVE muls + two PE matmuls + one ACT copy per 512-chunk (no serial
      3-op DVE chain). RoPE of head h+1 is interleaved into attention(j=0,h).
  attention j-outer/h-inner, transposed scores; exp on ACT -> pt bf16;
      softmax denominators accumulated in TWO independent accumulators
      (Pool-chain + DVE-chain) finished by two PSUM-accumulating
      ones-matmuls; causal diagonal tiles narrowed to valid columns.
  p3: o_proj matmuls for q-block j-1 interleaved into attention of q-block j
      (PE executes in program order: they fill ACT/Pool latency bubbles),
      ctx in SBUF bf16, w_o resident bf16.
"""
import sys

sys.path.insert(0, "/opt/trn_rl_repo")

import numpy as np

HIDDEN = 4096
N_Q_HEADS = 32
N_KV_HEADS = 4
HEAD_DIM = 128
ROPE_THETA = 500000.0
Q_SIZE = N_Q_HEADS * HEAD_DIM  # 4096
KV_SIZE = N_KV_HEADS * HEAD_DIM  # 512
B = 2
S = 2048
N_CORES = 8
N_GROUPS = 4
HEADS_PER_GROUP = N_Q_HEADS // N_GROUPS  # 8
GROUP_Q = HEADS_PER_GROUP * HEAD_DIM  # 1024
QKV_G = GROUP_Q + 2 * HEAD_DIM  # 1280 columns of qkv per group
SCALE = HEAD_DIM ** -0.5
NK = HIDDEN // 128  # 32 contraction k-tiles
NKT = S // 128  # 16 token/key tiles per sequence
NQB = S // 512  # 4 q-blocks
NHB = HIDDEN // 512  # 8 output-hidden blocks

_COMPILED = None
LAST_EXEC_NS = None


def _build(phases=(1, 2, 3)):
    import concourse.mybir as mybir
    import concourse.tile as tile
    from concourse import bacc

    F32 = mybir.dt.float32
    F32R = mybir.dt.float32r
    BF16 = mybir.dt.bfloat16

    nc = bacc.Bacc("TRN2", target_bir_lowering=False, debug=False, num_devices=N_CORES)

    # xt: [tt, 128, NK*128]; xt[tt, h, ko*128+t] = X[tt*128+t, ko*128+h]
    # (contiguous 8KB per partition row -> single-descriptor DMA lines)
    xt = nc.dram_tensor("xt", [NKT, 128, NK * 128], BF16, kind="ExternalInput").ap()
    # wqkvt: [128, NK, 1280]; wqkvt[p, ko, f] = W^T[ko*128+p, f], f=[k,v,q0..q7]
    wqkvt = nc.dram_tensor("wqkvt", [128, NK, QKV_G], BF16, kind="ExternalInput").ap()
    # wot: [128, hk, 4096]; wot[p, hk, o] = w_o[o, g*1024 + hk*128 + p]
    wot = nc.dram_tensor("wot", [128, HEADS_PER_GROUP, HIDDEN], BF16,
                         kind="ExternalInput").ap()
    # rope tables (bf16): cos2[2i]=cos2[2i+1]=cos; sin2[2i]=+sin, sin2[2i+1]=-sin
    cos2 = nc.dram_tensor("cos2", [HEAD_DIM, S], BF16, kind="ExternalInput").ap()
    sin2 = nc.dram_tensor("sin2", [HEAD_DIM, S], BF16, kind="ExternalInput").ap()
    swp = nc.dram_tensor("swp", [128, 128], BF16, kind="ExternalInput").ap()
    ones = nc.dram_tensor("ones", [128, 128], F32R, kind="ExternalInput").ap()
    ident = nc.dram_tensor("ident", [128, 128], BF16, kind="ExternalInput").ap()
    # triangular edge mask: maskt[p, c] = 1 if c >= p else 0
    maskt = nc.dram_tensor("maskt", [128, 128], BF16, kind="ExternalInput").ap()
    out_part = nc.dram_tensor(
        "out_part", [NKT, NHB, 128, 512], F32, kind="ExternalOutput"
    ).ap()

    FB = ((0, 256), (256, 512), (768, 512))  # (col0, width): kv | q0..3 | q4..7

    with tile.TileContext(nc) as tc:
        with (
            tc.tile_pool(name="consts", bufs=1) as cpool,
            tc.tile_pool(name="kvsb", bufs=1) as kvpool,
            tc.tile_pool(name="rqsb", bufs=1) as rqpool,
            tc.tile_pool(name="ropet", bufs=4) as ropet,
        ):
            swp_sb = cpool.tile([128, 128], BF16)
            ones_sb = cpool.tile([128, 128], F32R)
            id_sb = cpool.tile([128, 128], BF16)
            mask_sb = cpool.tile([128, 128], BF16)
            cos_sb = cpool.tile([128, S], BF16)
            sin_sb = cpool.tile([128, S], BF16)

            # persistent SBUF state
            kv_sb = [kvpool.tile([128, 256], BF16, name=f"kv{i}") for i in range(NKT)]
            rq = [rqpool.tile([128, S], BF16, name=f"rq{i}") for i in range(HEADS_PER_GROUP)]
            rk = rqpool.tile([128, S], BF16)

            def rope_chunk(t, c, pspool, pstag):
                # t[:, cs] = swp@(t*sin') + I@(t*cos), psum-accumulated
                cs = slice(c * 512, (c + 1) * 512)
                m_sin = ropet.tile([128, 512], BF16, tag="msin", name="msin")
                nc.vector.tensor_mul(m_sin, t[:, cs], sin_sb[:, cs])
                m_cos = ropet.tile([128, 512], BF16, tag="mcos", name="mcos")
                nc.vector.tensor_mul(m_cos, t[:, cs], cos_sb[:, cs])
                ps = pspool.tile([128, 512], F32, tag=pstag, name="ropeps")
                nc.tensor.matmul(ps, swp_sb, m_sin, start=True, stop=False)
                nc.tensor.matmul(ps, id_sb, m_cos, start=False, stop=True)
                nc.scalar.copy(t[:, cs], ps)

            def rope_inplace(t, pspool, pstag):
                for c in range(S // 512):
                    rope_chunk(t, c, pspool, pstag)

            # ---------------- phase 1: qkv + interleaved transposes ---------
            with (
                tc.tile_pool(name="w", bufs=1) as wpool,
                tc.tile_pool(name="x", bufs=5) as xpool,
                tc.tile_pool(name="qs", bufs=3) as qspool,
                tc.tile_pool(name="p1ps", bufs=3, space="PSUM") as p1ps,
                tc.tile_pool(name="tp", bufs=4, space="PSUM") as tpps,
            ):
                w_tiles = {}
                x_tiles = {}

                def emit_x(tt):
                    x_tile = xpool.tile([128, NK * 128], BF16, tag="xt", name="xtl")
                    nc.sync.dma_start(x_tile, xt[tt])
                    x_tiles[tt] = x_tile

                if 1 in phases:
                    # first W chunk, then x0, then the rest interleaved so the
                    # serial DMA device serves the critical path first
                    for fb, (c0, fw) in enumerate(FB):
                        for kc in range(4):
                            wt = wpool.tile([128, 8, fw], BF16, name=f"w{fb}_{kc}")
                            w_tiles[(fb, kc)] = wt

                    def wdma(fb, kc):
                        c0, fw = FB[fb]
                        nc.sync.dma_start(
                            w_tiles[(fb, kc)],
                            wqkvt[:, kc * 8:(kc + 1) * 8, c0:c0 + fw],
                        )

                    # first W chunk in two halves so the PE starts sooner
                    nc.sync.dma_start(
                        w_tiles[(0, 0)][:, :4, :], wqkvt[:, 0:4, 0:256]
                    )
                    # x0 in two halves so the first kv matmuls start sooner
                    x_tile0 = xpool.tile([128, NK * 128], BF16, tag="xt", name="xtl0")
                    nc.sync.dma_start(x_tile0[:, :1024], xt[0][:, :1024])
                    nc.sync.dma_start(
                        w_tiles[(0, 0)][:, 4:, :], wqkvt[:, 4:8, 0:256]
                    )
                    nc.sync.dma_start(x_tile0[:, 1024:2048], xt[0][:, 1024:2048])
                    wdma(0, 1)
                    nc.sync.dma_start(x_tile0[:, 2048:], xt[0][:, 2048:])
                    x_tiles[0] = x_tile0
                    wdma(0, 2), wdma(0, 3)
                    nc.sync.dma_start(id_sb, ident)
                    emit_x(1)
                    emit_x(2)
                    emit_x(3)
                    for kc in range(4):
                        wdma(1, kc)
                    for kc in range(4):
                        wdma(2, kc)
                    # remaining consts trickle in behind the p1-critical DMAs
                    nc.sync.dma_start(swp_sb, swp)
                    nc.sync.dma_start(ones_sb, ones)
                    nc.sync.dma_start(mask_sb, maskt)
                    nc.sync.dma_start(cos_sb, cos2)
                    nc.sync.dma_start(sin_sb, sin2)

                def transpose_to(src):
                    tps = tpps.tile([128, 128], BF16, tag="tp", name="tps")
                    nc.tensor.transpose(tps, src, id_sb)
                    return tps

                pend_q = []  # deferred q transposes: (qs_tile, hh_base, tt)

                def p1_block(tt, fb):
                    c0, fw = FB[fb]
                    ps = p1ps.tile([128, 512], F32, tag="p1", name="p1t")
                    for k in range(NK):
                        nc.tensor.matmul(
                            ps[:, :fw],
                            x_tiles[tt][:, k * 128:(k + 1) * 128],
                            w_tiles[(fb, k // 8)][:, k % 8, :],
                            start=(k == 0),
                            stop=(k == NK - 1),
                        )
                    if fb == 0:
                        nc.vector.tensor_copy(kv_sb[tt], ps[:, :256])
                    else:
                        qs = qspool.tile([128, 512], BF16, tag="qs", name="qst")
                        nc.vector.tensor_copy(qs, ps[:, :fw])
                        pend_q.append((qs, (fb - 1) * 4, tt))
                    if fb == 1:
                        # k transpose emitted while later matmuls fill PE
                        tps = transpose_to(kv_sb[tt][:, 0:128])
                        nc.scalar.copy(rk[:, tt * 128:(tt + 1) * 128], tps)
                    # drain one pending q-transpose batch per block,
                    # lagging behind the DVE copies
                    if len(pend_q) > 1:
                        qs_t, hh0, qtt = pend_q.pop(0)
                        for hh in range(4):
                            h = hh0 + hh
                            tps = transpose_to(qs_t[:, hh * 128:(hh + 1) * 128])
                            nc.scalar.copy(
                                rq[h][:, qtt * 128:(qtt + 1) * 128], tps
                            )

                if 1 in phases:
                    # first 4 token tiles feature-block-major: only w(fb0)
                    # gates the PE start while the rest of W streams in
                    for fb in range(3):
                        for tt in range(4):
                            p1_block(tt, fb)
                            if fb == 2:
                                emit_x(4 + tt)
                    rope_chunk_ok = 2 in phases
                    if rope_chunk_ok:
                        rope_chunk(rk, 0, p1ps, "p1")
                    for tt in range(4, NKT):
                        for fb in range(3):
                            p1_block(tt, fb)
                        if tt + 4 < NKT:
                            emit_x(tt + 4)
                        if tt % 4 == 3 and rope_chunk_ok:
                            rope_chunk(rk, tt // 4, p1ps, "p1")
                for qs_t, hh0, qtt in pend_q:
                    for hh in range(4):
                        h = hh0 + hh
                        tps = transpose_to(qs_t[:, hh * 128:(hh + 1) * 128])
                        nc.scalar.copy(rq[h][:, qtt * 128:(qtt + 1) * 128], tps)
                if 1 in phases and 2 in phases:
                    rope_inplace(rq[0], p1ps, "p1")

            # ---------------- phase 2/3: rope + attention + o_proj ----------
            with (
                tc.tile_pool(name="wo", bufs=1) as wopool,
                tc.tile_pool(name="pt", bufs=8) as ptpool,
                tc.tile_pool(name="pacc", bufs=4) as paccpool,
                tc.tile_pool(name="rcp", bufs=2) as rcppool,
                tc.tile_pool(name="ctxsb", bufs=2) as ctxsbpool,
                tc.tile_pool(name="ost", bufs=4) as ostpool,
                tc.tile_pool(name="scps", bufs=4, space="PSUM") as scps,
                tc.tile_pool(name="ctxps", bufs=2, space="PSUM") as ctxps,
                tc.tile_pool(name="opps", bufs=2, space="PSUM") as opps,
            ):
                wo_sb = wopool.tile([128, HEADS_PER_GROUP, HIDDEN], BF16)
                if 3 in phases:
                    for hb in range(NHB):
                        nc.sync.dma_start(
                            wo_sb[:, :, hb * 512:(hb + 1) * 512],
                            wot[:, :, hb * 512:(hb + 1) * 512],
                        )

                # o_proj work items for q-block j, emitted interleaved during
                # attention of q-block j+1 (fills PE while ACT/Pool run)
                class OpEmitter:
                    def __init__(self, j, ctx_sb_j):
                        self.items = [
                            (tl, hb, hk)
                            for tl in range(4)
                            for hb in range(NHB)
                            for hk in range(HEADS_PER_GROUP)
                        ] if (3 in phases) else []
                        self.j = j
                        self.ctx = ctx_sb_j
                        self.pos = 0
                        self.ps = None

                    def emit(self, n):
                        for _ in range(n):
                            if self.pos >= len(self.items):
                                return
                            tl, hb, hk = self.items[self.pos]
                            if hk == 0:
                                self.ps = opps.tile([128, 512], F32, tag="op", name="opps")
                            nc.tensor.matmul(
                                self.ps,
                                self.ctx[:, hk, tl * 128:(tl + 1) * 128],
                                wo_sb[:, hk, hb * 512:(hb + 1) * 512],
                                start=(hk == 0),
                                stop=(hk == HEADS_PER_GROUP - 1),
                            )
                            if hk == HEADS_PER_GROUP - 1:
                                ost = ostpool.tile([128, 512], F32, tag="ost", name="ost")
                                if (self.pos // 8) % 2 == 0:
                                    nc.vector.tensor_copy(ost, self.ps)
                                else:
                                    nc.scalar.copy(ost, self.ps)
                                nc.sync.dma_start(
                                    out_part[self.j * 4 + tl, hb], ost
                                )
                            self.pos += 1

                    def flush(self):
                        self.emit(len(self.items) - self.pos)

                def finalize(fin):
                    pacc_a, pacc_b, ctx_ps, ctx_dst = fin
                    acc = pacc_a
                    if pacc_b is not None:
                        nc.vector.tensor_add(
                            pacc_a, pacc_a.bitcast(F32), pacc_b.bitcast(F32)
                        )
                    r_ps = scps.tile([128, 512], F32, tag="sc", name="rpst")
                    nc.tensor.matmul(r_ps, ones_sb, acc, start=True, stop=True)
                    rcp = rcppool.tile([128, 512], F32, tag="rcp", name="rcpt")
                    nc.vector.reciprocal(rcp, r_ps)
                    nc.vector.tensor_mul(ctx_dst, ctx_ps, rcp)

                prev_op = None
                fin = None
                for j in range(NQB if 2 in phases else 0):
                    nkt_j = 4 * (j + 1)
                    # diagonal tiles first (descending width), then full tiles
                    kt_order = list(range(4 * j, 4 * j + 4)) + list(range(4 * j))
                    ctx_sb_j = ctxsbpool.tile(
                        [128, HEADS_PER_GROUP, 512], BF16, tag="ctx", name="ctxsb"
                    )
                    op_budget = 0.0
                    op_step = (32.0 / nkt_j) if prev_op is not None else 0.0
                    for h in range(HEADS_PER_GROUP):
                        ctx_ps = ctxps.tile([128, 512], F32, tag="ctxp", name="ctxpt")
                        pacc_a = paccpool.tile([128, 512], F32R, tag="pacca", name="pacca")
                        pacc_b = None
                        b_init = False
                        flip = False

                        def sc_emit(i):
                            kt = kt_order[i]
                            di = kt - 4 * j
                            col0 = di * 128 if di >= 0 else 0
                            sc_ps = scps.tile([128, 512], F32, tag="sc", name="scpst")
                            nc.tensor.matmul(
                                sc_ps[:, col0:],
                                rk[:, kt * 128:(kt + 1) * 128],
                                rq[h][:, j * 512 + col0:(j + 1) * 512],
                                start=True,
                                stop=True,
                            )
                            return sc_ps, kt, col0

                        pend = [sc_emit(0)]
                        if nkt_j > 1:
                            pend.append(sc_emit(1))
                        if nkt_j > 2:
                            pend.append(sc_emit(2))
                        if fin is not None:
                            finalize(fin)
                            fin = None
                        for i in range(nkt_j):
                            sc_ps, kt, col0 = pend.pop(0)
                            di = kt - 4 * j
                            pt = ptpool.tile([128, 512], BF16, tag="pt", name="ptt")
                            nc.scalar.activation(
                                pt[:, col0:], sc_ps[:, col0:],
                                mybir.ActivationFunctionType.Exp,
                                scale=SCALE,
                            )
                            if di >= 0:  # diagonal: mask triangular edge
                                nc.vector.tensor_mul(
                                    pt[:, col0:col0 + 128],
                                    pt[:, col0:col0 + 128],
                                    mask_sb,
                                )
                            # dual-chain denominator accumulation
                            if i == 0:
                                nc.gpsimd.tensor_copy(pacc_a, pt)
                            elif col0 == 0 and not b_init:
                                pacc_b = paccpool.tile(
                                    [128, 512], F32R, tag="paccb", name="paccb"
                                )
                                nc.vector.tensor_copy(pacc_b, pt)
                                b_init = True
                            elif (not b_init) or flip:
                                nc.gpsimd.tensor_add(
                                    pacc_a[:, col0:],
                                    pacc_a[:, col0:].bitcast(F32),
                                    pt[:, col0:],
                                )
                                flip = False
                            else:
                                nc.vector.tensor_add(
                                    pacc_b[:, col0:],
                                    pacc_b[:, col0:].bitcast(F32),
                                    pt[:, col0:],
                                )
                                flip = True
                            if prev_op is not None:
                                op_budget += op_step
                                n_emit = int(op_budget)
                                op_budget -= n_emit
                                prev_op.emit(n_emit)
                            nc.tensor.matmul(
                                ctx_ps[:, col0:],
                                kv_sb[kt][:, 128:256],
                                pt[:, col0:],
                                start=(i == 0),
                                stop=(i == nkt_j - 1),
                                skip_group_check=True,
                            )
                            if i + 3 < nkt_j:
                                pend.append(sc_emit(i + 3))
                        fin = (pacc_a, pacc_b, ctx_ps, ctx_sb_j[:, h, :])
                        # interleave rope of the next head into attention(j=0)
                        if j == 0 and h + 1 < HEADS_PER_GROUP:
                            rope_inplace(rq[h + 1], scps, "sc")
                    if prev_op is not None:
                        prev_op.flush()
                    prev_op = OpEmitter(j, ctx_sb_j)
                if fin is not None:
                    finalize(fin)
                    fin = None
                if prev_op is not None:
                    prev_op.flush()

    nc.compile()
    return nc


def _host_inputs(positions, hidden_states, w_qkv, w_o):
    """Shard + lay out inputs for the 8 cores (c = 4*b + g)."""
    import ml_dtypes

    bf16 = ml_dtypes.bfloat16
    positions = np.asarray(positions)
    hidden_states = np.asarray(hidden_states, dtype=np.float32)
    w_qkv = np.asarray(w_qkv, dtype=np.float32)
    w_o = np.asarray(w_o, dtype=np.float32)

    inv_freq = 1.0 / (ROPE_THETA ** (np.arange(0, HEAD_DIM, 2, dtype=np.float64) / HEAD_DIM))
    ang = positions.astype(np.float64)[None, :] * inv_freq[:, None]  # [half, S]
    c = np.cos(ang).astype(np.float32)
    s = np.sin(ang).astype(np.float32)
    cos2 = np.empty((HEAD_DIM, S), dtype=np.float32)
    sin2 = np.empty((HEAD_DIM, S), dtype=np.float32)
    cos2[0::2] = c
    cos2[1::2] = c
    sin2[0::2] = s
    sin2[1::2] = -s

    swp = np.zeros((128, 128), dtype=np.float32)
    idx = np.arange(0, 128, 2)
    swp[idx, idx + 1] = 1.0
    swp[idx + 1, idx] = 1.0
    ones = np.ones((128, 128), dtype=np.float32)
    ident = np.eye(128, dtype=np.float32)
    maskt = (np.arange(128)[None, :] >= np.arange(128)[:, None]).astype(np.float32)

    xts = []
    for b in range(B):
        xt_t = np.ascontiguousarray(
            hidden_states[b].reshape(NKT, 128, NK, 128).transpose(0, 3, 2, 1)
        ).reshape(NKT, 128, NK * 128).astype(bf16)
        xts.append(xt_t)

    in_maps = []
    for c_id in range(N_CORES):
        b, g = divmod(c_id, N_GROUPS)
        cols = np.concatenate([
            np.arange(Q_SIZE + g * HEAD_DIM, Q_SIZE + (g + 1) * HEAD_DIM),  # k
            np.arange(Q_SIZE + KV_SIZE + g * HEAD_DIM, Q_SIZE + KV_SIZE + (g + 1) * HEAD_DIM),  # v
            np.arange(g * GROUP_Q, (g + 1) * GROUP_Q),  # q0..q7
        ])
        wq = w_qkv[cols, :]  # [1280, 4096]
        wqkvt_t = np.ascontiguousarray(
            wq.T.reshape(NK, 128, QKV_G).transpose(1, 0, 2)
        ).astype(bf16)
        wot_full = w_o[:, g * GROUP_Q:(g + 1) * GROUP_Q].T  # [1024, 4096]
        wot_t = np.ascontiguousarray(
            wot_full.reshape(HEADS_PER_GROUP, 128, HIDDEN).transpose(1, 0, 2)
        ).astype(bf16)
        in_maps.append({
            "xt": xts[b],
            "wqkvt": wqkvt_t,
            "wot": wot_t,
            "cos2": cos2.astype(bf16),
            "sin2": sin2.astype(bf16),
            "swp": swp.astype(bf16),
            "ones": ones,
            "ident": ident.astype(bf16),
            "maskt": maskt.astype(bf16),
        })
    return in_maps


def kernel(positions, hidden_states, w_qkv, w_o):
    global _COMPILED, LAST_EXEC_NS
    from concourse import bass_utils

    if _COMPILED is None:
        _COMPILED = _build()
    nc = _COMPILED

    in_maps = _host_inputs(positions, hidden_states, w_qkv, w_o)
    res = bass_utils.run_bass_kernel_spmd(
        nc, in_maps, core_ids=list(range(N_CORES))
    )
    LAST_EXEC_NS = res.exec_time_ns

    out = np.zeros((B, S, HIDDEN), dtype=np.float32)
    for c_id in range(N_CORES):
        b = c_id // N_GROUPS
        part = res.results[c_id]["out_part"]  # [NKT, NHB, 128, 512]
        out[b] += part.transpose(0, 2, 1, 3).reshape(S, HIDDEN)
    return out


# revision 17
# speedup vs baseline: 1.3404x; 1.0020x over previous
"""Ernie4 GQA attention layer as a Bass/Tile kernel for 8 TRN2 NeuronCores (v3).

Sharding: core c = 4*b + g handles batch b (of 2) and head-group g (of 4).
Each group owns 8 query heads + 1 kv head (GQA 32q/4kv, head_dim 128) and the
matching column slice of w_qkv / row slice of w_o. The o_proj partial sums are
reduced on the host (all-reduce equivalent).

v3 design: everything SBUF-resident in bf16 (PE rate equals f32r at 1
cycle/row but SBUF/DMA halve), no DRAM scratch round-trips.
  p1: qkv = X^T.T @ W^T token-major, 3 feature blocks (kv 256 | q 512 | q 512)
      per token tile. X rows loaded as contiguous 8KB lines; W chunk DMAs
      interleaved with the first X tiles so the PE starts after ~4us.
      Per-head PE transposes interleaved into the p1 loop.
  p2: RoPE via PSUM accumulation: q' = swp@(q*sin') + I@(q*cos) — two cheap
      bf16 DVE muls + two PE matmuls + one ACT copy per 512-chunk (no serial
      3-op DVE chain). RoPE of head h+1 is interleaved into attention(j=0,h).
  attention j-outer/h-inner, transposed scores; exp on ACT -> pt bf16;
      softmax denominators accumulated in TWO independent accumulators
      (Pool-chain + DVE-chain) finished by two PSUM-accumulating
      ones-matmuls; causal diagonal tiles narrowed to valid columns.
  p3: o_proj matmuls for q-block j-1 interleaved into attention of q-block j
      (PE executes in program order: they fill ACT/Pool latency bubbles),
      ctx in SBUF bf16, w_o resident bf16.
"""
import sys

sys.path.insert(0, "/opt/trn_rl_repo")

import numpy as np

HIDDEN = 4096
N_Q_HEADS = 32
N_KV_HEADS = 4
HEAD_DIM = 128
ROPE_THETA = 500000.0
Q_SIZE = N_Q_HEADS * HEAD_DIM  # 4096
KV_SIZE = N_KV_HEADS * HEAD_DIM  # 512
B = 2
S = 2048
N_CORES = 8
N_GROUPS = 4
HEADS_PER_GROUP = N_Q_HEADS // N_GROUPS  # 8
GROUP_Q = HEADS_PER_GROUP * HEAD_DIM  # 1024
QKV_G = GROUP_Q + 2 * HEAD_DIM  # 1280 columns of qkv per group
SCALE = HEAD_DIM ** -0.5
NK = HIDDEN // 128  # 32 contraction k-tiles
NKT = S // 128  # 16 token/key tiles per sequence
NQB = S // 512  # 4 q-blocks
NHB = HIDDEN // 512  # 8 output-hidden blocks

_COMPILED = None
LAST_EXEC_NS = None


def _build(phases=(1, 2, 3)):
    import concourse.mybir as mybir
    import concourse.tile as tile
    from concourse import bacc

    F32 = mybir.dt.float32
    F32R = mybir.dt.float32r
    BF16 = mybir.dt.bfloat16

    nc = bacc.Bacc("TRN2", target_bir_lowering=False, debug=False, num_devices=N_CORES)

    # xt: [tt, 128, NK*128]; xt[tt, h, ko*128+t] = X[tt*128+t, ko*128+h]
    # (contiguous 8KB per partition row -> single-descriptor DMA lines)
    xt = nc.dram_tensor("xt", [NKT, 128, NK * 128], BF16, kind="ExternalInput").ap()
    # wqkvt: [128, NK, 1280]; wqkvt[p, ko, f] = W^T[ko*128+p, f], f=[k,v,q0..q7]
    wqkvt = nc.dram_tensor("wqkvt", [128, NK, QKV_G], BF16, kind="ExternalInput").ap()
    # wot: [128, hk, 4096]; wot[p, hk, o] = w_o[o, g*1024 + hk*128 + p]
    wot = nc.dram_tensor("wot", [128, HEADS_PER_GROUP, HIDDEN], BF16,
                         kind="ExternalInput").ap()
    # rope tables (bf16): cos2[2i]=cos2[2i+1]=cos; sin2[2i]=+sin, sin2[2i+1]=-sin
    cos2 = nc.dram_tensor("cos2", [HEAD_DIM, S], BF16, kind="ExternalInput").ap()
    sin2 = nc.dram_tensor("sin2", [HEAD_DIM, S], BF16, kind="ExternalInput").ap()
    swp = nc.dram_tensor("swp", [128, 128], BF16, kind="ExternalInput").ap()
    ones = nc.dram_tensor("ones", [128, 128], F32R, kind="ExternalInput").ap()
    ident = nc.dram_tensor("ident", [128, 128], BF16, kind="ExternalInput").ap()
    # triangular edge mask: maskt[p, c] = 1 if c >= p else 0
    maskt = nc.dram_tensor("maskt", [128, 128], BF16, kind="ExternalInput").ap()
    out_part = nc.dram_tensor(
        "out_part", [NKT, NHB, 128, 512], F32, kind="ExternalOutput"
    ).ap()

    FB = ((0, 256), (256, 512), (768, 512))  # (col0, width): kv | q0..3 | q4..7

    with tile.TileContext(nc) as tc:
        with (
            tc.tile_pool(name="consts", bufs=1) as cpool,
            tc.tile_pool(name="kvsb", bufs=1) as kvpool,
            tc.tile_pool(name="rqsb", bufs=1) as rqpool,
            tc.tile_pool(name="ropet", bufs=4) as ropet,
        ):
            swp_sb = cpool.tile([128, 128], BF16)
            ones_sb = cpool.tile([128, 128], F32R)
            id_sb = cpool.tile([128, 128], BF16)
            mask_sb = cpool.tile([128, 128], BF16)
            cos_sb = cpool.tile([128, S], BF16)
            sin_sb = cpool.tile([128, S], BF16)

            # persistent SBUF state
            kv_sb = [kvpool.tile([128, 256], BF16, name=f"kv{i}") for i in range(NKT)]
            rq = [rqpool.tile([128, S], BF16, name=f"rq{i}") for i in range(HEADS_PER_GROUP)]
            rk = rqpool.tile([128, S], BF16)

            def rope_chunk(t, c, pspool, pstag, dve_add=False, copy_dve=False):
                # t[:, cs] = swp@(t*sin') + I@(t*cos), psum-accumulated
                cs = slice(c * 512, (c + 1) * 512)
                m_sin = ropet.tile([128, 512], BF16, tag="msin", name="msin")
                nc.vector.tensor_mul(m_sin, t[:, cs], sin_sb[:, cs])
                m_cos = ropet.tile([128, 512], BF16, tag="mcos", name="mcos")
                nc.vector.tensor_mul(m_cos, t[:, cs], cos_sb[:, cs])
                ps = pspool.tile([128, 512], F32, tag=pstag, name="ropeps")
                if dve_add:
                    # phase-1 form: DVE (idle there) does the final add
                    nc.tensor.matmul(ps, swp_sb, m_sin, start=True, stop=True)
                    nc.vector.tensor_add(t[:, cs], ps, m_cos)
                    return
                nc.tensor.matmul(ps, swp_sb, m_sin, start=True, stop=False)
                nc.tensor.matmul(ps, id_sb, m_cos, start=False, stop=True)
                if copy_dve:
                    nc.vector.tensor_copy(t[:, cs], ps)
                else:
                    nc.scalar.copy(t[:, cs], ps)

            def rope_inplace(t, pspool, pstag, dve_add=False):
                for c in range(S // 512):
                    rope_chunk(t, c, pspool, pstag, dve_add=dve_add)

            # ---------------- phase 1: qkv + interleaved transposes ---------
            with (
                tc.tile_pool(name="w", bufs=1) as wpool,
                tc.tile_pool(name="x", bufs=5) as xpool,
                tc.tile_pool(name="qs", bufs=3) as qspool,
                tc.tile_pool(name="p1ps", bufs=3, space="PSUM") as p1ps,
                tc.tile_pool(name="tp", bufs=4, space="PSUM") as tpps,
            ):
                w_tiles = {}
                x_tiles = {}

                def emit_x(tt):
                    x_tile = xpool.tile([128, NK * 128], BF16, tag="xt", name="xtl")
                    nc.sync.dma_start(x_tile, xt[tt])
                    x_tiles[tt] = x_tile

                if 1 in phases:
                    # first W chunk, then x0, then the rest interleaved so the
                    # serial DMA device serves the critical path first
                    for fb, (c0, fw) in enumerate(FB):
                        for kc in range(4):
                            wt = wpool.tile([128, 8, fw], BF16, name=f"w{fb}_{kc}")
                            w_tiles[(fb, kc)] = wt

                    def wdma(fb, kc):
                        c0, fw = FB[fb]
                        nc.sync.dma_start(
                            w_tiles[(fb, kc)],
                            wqkvt[:, kc * 8:(kc + 1) * 8, c0:c0 + fw],
                        )

                    # first W chunk in two halves so the PE starts sooner
                    nc.sync.dma_start(
                        w_tiles[(0, 0)][:, :4, :], wqkvt[:, 0:4, 0:256]
                    )
                    # x0 in two halves so the first kv matmuls start sooner
                    x_tile0 = xpool.tile([128, NK * 128], BF16, tag="xt", name="xtl0")
                    nc.sync.dma_start(x_tile0[:, :1024], xt[0][:, :1024])
                    nc.sync.dma_start(
                        w_tiles[(0, 0)][:, 4:, :], wqkvt[:, 4:8, 0:256]
                    )
                    nc.sync.dma_start(x_tile0[:, 1024:2048], xt[0][:, 1024:2048])
                    wdma(0, 1)
                    nc.sync.dma_start(x_tile0[:, 2048:], xt[0][:, 2048:])
                    x_tiles[0] = x_tile0
                    wdma(0, 2), wdma(0, 3)
                    nc.sync.dma_start(id_sb, ident)
                    emit_x(1)
                    emit_x(2)
                    emit_x(3)
                    for kc in range(4):
                        wdma(1, kc)
                    for kc in range(4):
                        wdma(2, kc)
                    # remaining consts trickle in behind the p1-critical DMAs
                    nc.sync.dma_start(swp_sb, swp)
                    nc.sync.dma_start(ones_sb, ones)
                    nc.sync.dma_start(mask_sb, maskt)
                    nc.sync.dma_start(cos_sb, cos2)
                    nc.sync.dma_start(sin_sb, sin2)

                def transpose_to(src):
                    tps = tpps.tile([128, 128], BF16, tag="tp", name="tps")
                    nc.tensor.transpose(tps, src, id_sb)
                    return tps

                pend_q = []  # deferred q transposes: (qs_tile, hh_base, tt)

                def p1_block(tt, fb):
                    c0, fw = FB[fb]
                    ps = p1ps.tile([128, 512], F32, tag="p1", name="p1t")
                    for k in range(NK):
                        nc.tensor.matmul(
                            ps[:, :fw],
                            x_tiles[tt][:, k * 128:(k + 1) * 128],
                            w_tiles[(fb, k // 8)][:, k % 8, :],
                            start=(k == 0),
                            stop=(k == NK - 1),
                        )
                    if fb == 0:
                        nc.vector.tensor_copy(kv_sb[tt], ps[:, :256])
                    else:
                        qs = qspool.tile([128, 512], BF16, tag="qs", name="qst")
                        nc.vector.tensor_copy(qs, ps[:, :fw])
                        pend_q.append((qs, (fb - 1) * 4, tt))
                    if fb == 1:
                        # k transpose emitted while later matmuls fill PE
                        tps = transpose_to(kv_sb[tt][:, 0:128])
                        nc.scalar.copy(rk[:, tt * 128:(tt + 1) * 128], tps)
                    # drain one pending q-transpose batch per block,
                    # lagging behind the DVE copies
                    if len(pend_q) > 1:
                        qs_t, hh0, qtt = pend_q.pop(0)
                        for hh in range(4):
                            h = hh0 + hh
                            tps = transpose_to(qs_t[:, hh * 128:(hh + 1) * 128])
                            nc.scalar.copy(
                                rq[h][:, qtt * 128:(qtt + 1) * 128], tps
                            )

                if 1 in phases:
                    # first 4 token tiles feature-block-major: only w(fb0)
                    # gates the PE start while the rest of W streams in
                    for fb in range(3):
                        for tt in range(4):
                            p1_block(tt, fb)
                            if fb == 2:
                                emit_x(4 + tt)
                    rope_chunk_ok = 2 in phases
                    if rope_chunk_ok:
                        rope_chunk(rk, 0, p1ps, "p1", dve_add=True)
                    for tt in range(4, NKT):
                        for fb in range(3):
                            p1_block(tt, fb)
                        if tt + 4 < NKT:
                            emit_x(tt + 4)
                        if tt % 4 == 3 and rope_chunk_ok:
                            rope_chunk(rk, tt // 4, p1ps, "p1", dve_add=True)
                for qs_t, hh0, qtt in pend_q:
                    for hh in range(4):
                        h = hh0 + hh
                        tps = transpose_to(qs_t[:, hh * 128:(hh + 1) * 128])
                        nc.scalar.copy(rq[h][:, qtt * 128:(qtt + 1) * 128], tps)
                if 1 in phases and 2 in phases:
                    rope_inplace(rq[0], p1ps, "p1", dve_add=True)

            # ---------------- phase 2/3: rope + attention + o_proj ----------
            with (
                tc.tile_pool(name="wo", bufs=1) as wopool,
                tc.tile_pool(name="pt", bufs=8) as ptpool,
                tc.tile_pool(name="pacc", bufs=4) as paccpool,
                tc.tile_pool(name="rcp", bufs=2) as rcppool,
                tc.tile_pool(name="ctxsb", bufs=2) as ctxsbpool,
                tc.tile_pool(name="ost", bufs=4) as ostpool,
                tc.tile_pool(name="scps", bufs=4, space="PSUM") as scps,
                tc.tile_pool(name="ctxps", bufs=2, space="PSUM") as ctxps,
                tc.tile_pool(name="opps", bufs=2, space="PSUM") as opps,
            ):
                wo_sb = wopool.tile([128, HEADS_PER_GROUP, HIDDEN], BF16)
                if 3 in phases:
                    for hb in range(NHB):
                        nc.sync.dma_start(
                            wo_sb[:, :, hb * 512:(hb + 1) * 512],
                            wot[:, :, hb * 512:(hb + 1) * 512],
                        )

                # o_proj work items for q-block j, emitted interleaved during
                # attention of q-block j+1 (fills PE while ACT/Pool run)
                class OpEmitter:
                    def __init__(self, j, ctx_sb_j):
                        self.items = [
                            (tl, hb, hk)
                            for tl in range(4)
                            for hb in range(NHB)
                            for hk in range(HEADS_PER_GROUP)
                        ] if (3 in phases) else []
                        self.j = j
                        self.ctx = ctx_sb_j
                        self.pos = 0
                        self.ps = None

                    def emit(self, n):
                        for _ in range(n):
                            if self.pos >= len(self.items):
                                return
                            tl, hb, hk = self.items[self.pos]
                            if hk == 0:
                                self.ps = opps.tile([128, 512], F32, tag="op", name="opps")
                            nc.tensor.matmul(
                                self.ps,
                                self.ctx[:, hk, tl * 128:(tl + 1) * 128],
                                wo_sb[:, hk, hb * 512:(hb + 1) * 512],
                                start=(hk == 0),
                                stop=(hk == HEADS_PER_GROUP - 1),
                            )
                            if hk == HEADS_PER_GROUP - 1:
                                ost = ostpool.tile([128, 512], F32, tag="ost", name="ost")
                                if (self.pos // 8) % 2 == 0:
                                    nc.vector.tensor_copy(ost, self.ps)
                                else:
                                    nc.scalar.copy(ost, self.ps)
                                nc.sync.dma_start(
                                    out_part[self.j * 4 + tl, hb], ost
                                )
                            self.pos += 1

                    def flush(self):
                        self.emit(len(self.items) - self.pos)

                def finalize(fin):
                    pacc_a, pacc_b, ctx_ps, ctx_dst = fin
                    acc = pacc_a
                    if pacc_b is not None:
                        nc.vector.tensor_add(
                            pacc_a, pacc_a.bitcast(F32), pacc_b.bitcast(F32)
                        )
                    r_ps = scps.tile([128, 512], F32, tag="sc", name="rpst")
                    nc.tensor.matmul(r_ps, ones_sb, acc, start=True, stop=True)
                    rcp = rcppool.tile([128, 512], F32, tag="rcp", name="rcpt")
                    nc.vector.reciprocal(rcp, r_ps)
                    nc.vector.tensor_mul(ctx_dst, ctx_ps, rcp)

                prev_op = None
                fin = None
                for j in range(NQB if 2 in phases else 0):
                    nkt_j = 4 * (j + 1)
                    # diagonal tiles first (descending width), then full tiles
                    kt_order = list(range(4 * j, 4 * j + 4)) + list(range(4 * j))
                    ctx_sb_j = ctxsbpool.tile(
                        [128, HEADS_PER_GROUP, 512], BF16, tag="ctx", name="ctxsb"
                    )
                    op_budget = 0.0
                    op_step = (32.0 / nkt_j) if prev_op is not None else 0.0
                    for h in range(HEADS_PER_GROUP):
                        ctx_ps = ctxps.tile([128, 512], F32, tag="ctxp", name="ctxpt")
                        pacc_a = paccpool.tile([128, 512], F32R, tag="pacca", name="pacca")
                        pacc_b = None
                        b_init = False
                        flip = False

                        def sc_emit(i):
                            kt = kt_order[i]
                            di = kt - 4 * j
                            col0 = di * 128 if di >= 0 else 0
                            sc_ps = scps.tile([128, 512], F32, tag="sc", name="scpst")
                            nc.tensor.matmul(
                                sc_ps[:, col0:],
                                rk[:, kt * 128:(kt + 1) * 128],
                                rq[h][:, j * 512 + col0:(j + 1) * 512],
                                start=True,
                                stop=True,
                            )
                            return sc_ps, kt, col0

                        pend = [sc_emit(0)]
                        if nkt_j > 1:
                            pend.append(sc_emit(1))
                        if nkt_j > 2:
                            pend.append(sc_emit(2))
                        if fin is not None:
                            finalize(fin)
                            fin = None
                        for i in range(nkt_j):
                            sc_ps, kt, col0 = pend.pop(0)
                            di = kt - 4 * j
                            pt = ptpool.tile([128, 512], BF16, tag="pt", name="ptt")
                            nc.scalar.activation(
                                pt[:, col0:], sc_ps[:, col0:],
                                mybir.ActivationFunctionType.Exp,
                                scale=SCALE,
                            )
                            if di >= 0:  # diagonal: mask triangular edge
                                nc.vector.tensor_mul(
                                    pt[:, col0:col0 + 128],
                                    pt[:, col0:col0 + 128],
                                    mask_sb,
                                )
                            # dual-chain denominator accumulation
                            if i == 0:
                                nc.gpsimd.tensor_copy(pacc_a, pt)
                            elif col0 == 0 and not b_init:
                                pacc_b = paccpool.tile(
                                    [128, 512], F32R, tag="paccb", name="paccb"
                                )
                                nc.vector.tensor_copy(pacc_b, pt)
                                b_init = True
                            elif (not b_init) or flip:
                                nc.gpsimd.tensor_add(
                                    pacc_a[:, col0:],
                                    pacc_a[:, col0:].bitcast(F32),
                                    pt[:, col0:],
                                )
                                flip = False
                            else:
                                nc.vector.tensor_add(
                                    pacc_b[:, col0:],
                                    pacc_b[:, col0:].bitcast(F32),
                                    pt[:, col0:],
                                )
                                flip = True
                            if prev_op is not None:
                                op_budget += op_step
                                n_emit = int(op_budget)
                                op_budget -= n_emit
                                prev_op.emit(n_emit)
                            nc.tensor.matmul(
                                ctx_ps[:, col0:],
                                kv_sb[kt][:, 128:256],
                                pt[:, col0:],
                                start=(i == 0),
                                stop=(i == nkt_j - 1),
                                skip_group_check=True,
                            )
                            if i + 3 < nkt_j:
                                pend.append(sc_emit(i + 3))
                        fin = (pacc_a, pacc_b, ctx_ps, ctx_sb_j[:, h, :])
                        # interleave rope of the next head into attention(j=0)
                        if j == 0 and h + 1 < HEADS_PER_GROUP:
                            rope_inplace(rq[h + 1], scps, "sc")
                    if prev_op is not None:
                        prev_op.flush()
                    prev_op = OpEmitter(j, ctx_sb_j)
                if fin is not None:
                    finalize(fin)
                    fin = None
                if prev_op is not None:
                    prev_op.flush()

    nc.compile()
    return nc


def _host_inputs(positions, hidden_states, w_qkv, w_o):
    """Shard + lay out inputs for the 8 cores (c = 4*b + g)."""
    import ml_dtypes

    bf16 = ml_dtypes.bfloat16
    positions = np.asarray(positions)
    hidden_states = np.asarray(hidden_states, dtype=np.float32)
    w_qkv = np.asarray(w_qkv, dtype=np.float32)
    w_o = np.asarray(w_o, dtype=np.float32)

    inv_freq = 1.0 / (ROPE_THETA ** (np.arange(0, HEAD_DIM, 2, dtype=np.float64) / HEAD_DIM))
    ang = positions.astype(np.float64)[None, :] * inv_freq[:, None]  # [half, S]
    c = np.cos(ang).astype(np.float32)
    s = np.sin(ang).astype(np.float32)
    cos2 = np.empty((HEAD_DIM, S), dtype=np.float32)
    sin2 = np.empty((HEAD_DIM, S), dtype=np.float32)
    cos2[0::2] = c
    cos2[1::2] = c
    sin2[0::2] = s
    sin2[1::2] = -s

    swp = np.zeros((128, 128), dtype=np.float32)
    idx = np.arange(0, 128, 2)
    swp[idx, idx + 1] = 1.0
    swp[idx + 1, idx] = 1.0
    ones = np.ones((128, 128), dtype=np.float32)
    ident = np.eye(128, dtype=np.float32)
    maskt = (np.arange(128)[None, :] >= np.arange(128)[:, None]).astype(np.float32)

    xts = []
    for b in range(B):
        xt_t = np.ascontiguousarray(
            hidden_states[b].reshape(NKT, 128, NK, 128).transpose(0, 3, 2, 1)
        ).reshape(NKT, 128, NK * 128).astype(bf16)
        xts.append(xt_t)

    in_maps = []
    for c_id in range(N_CORES):
        b, g = divmod(c_id, N_GROUPS)
        cols = np.concatenate([
            np.arange(Q_SIZE + g * HEAD_DIM, Q_SIZE + (g + 1) * HEAD_DIM),  # k
            np.arange(Q_SIZE + KV_SIZE + g * HEAD_DIM, Q_SIZE + KV_SIZE + (g + 1) * HEAD_DIM),  # v
            np.arange(g * GROUP_Q, (g + 1) * GROUP_Q),  # q0..q7
        ])
        wq = w_qkv[cols, :]  # [1280, 4096]
        wqkvt_t = np.ascontiguousarray(
            wq.T.reshape(NK, 128, QKV_G).transpose(1, 0, 2)
        ).astype(bf16)
        wot_full = w_o[:, g * GROUP_Q:(g + 1) * GROUP_Q].T  # [1024, 4096]
        wot_t = np.ascontiguousarray(
            wot_full.reshape(HEADS_PER_GROUP, 128, HIDDEN).transpose(1, 0, 2)
        ).astype(bf16)
        in_maps.append({
            "xt": xts[b],
            "wqkvt": wqkvt_t,
            "wot": wot_t,
            "cos2": cos2.astype(bf16),
            "sin2": sin2.astype(bf16),
            "swp": swp.astype(bf16),
            "ones": ones,
            "ident": ident.astype(bf16),
            "maskt": maskt.astype(bf16),
        })
    return in_maps


def kernel(positions, hidden_states, w_qkv, w_o):
    global _COMPILED, LAST_EXEC_NS
    from concourse import bass_utils

    if _COMPILED is None:
        _COMPILED = _build()
    nc = _COMPILED

    in_maps = _host_inputs(positions, hidden_states, w_qkv, w_o)
    res = bass_utils.run_bass_kernel_spmd(
        nc, in_maps, core_ids=list(range(N_CORES))
    )
    LAST_EXEC_NS = res.exec_time_ns

    out = np.zeros((B, S, HIDDEN), dtype=np.float32)
    for c_id in range(N_CORES):
        b = c_id // N_GROUPS
        part = res.results[c_id]["out_part"]  # [NKT, NHB, 128, 512]
        out[b] += part.transpose(0, 2, 1, 3).reshape(S, HIDDEN)
    return out
